# revision 4
# baseline (speedup 1.0000x reference)
"""Per-sample modulated conv2d (StyleGAN2-style Conv2dMod) on 8 trn2 NeuronCores.

Reference computation (fp32):
    scale[n,o] = (1+y[n,o]) * rsqrt(||W[o]||^2 * (1+y[n,o])^2 + 1e-8)
    out = conv2d(edge_pad(x), W) * scale[:, :, None, None]

Strategy: 1D Winograd F(2,3) along W + direct 3-tap convolution along H,
in bf16 (rel err ~3e-3, gate is 2e-2).  This cuts PE work 1.5x vs direct
conv: per-core matmul stream is 98304 cycles = 41 us @ 2.4 GHz instead of
147456 = 61.4 us, which is the direct-conv floor.

Sharding: 8 cores = 4 sample-pairs x 2 output-channel halves.  Core c
handles samples {2*(c//2), 2*(c//2)+1} and out channels
[256*(c%2), 256*(c%2)+256).  The oc split halves per-core weight DMA.

Per-core pipeline:
  - host uploads x edge-padded, column-deinterleaved (xe = even cols,
    xo = odd cols, bf16).  The deinterleave keeps every DVE transform
    operand innermost-contiguous so tensor_tensor runs in 2x_1p mode.
  - DVE computes the F(2,3) data transform V[pw] per (sample, ic chunk):
      V0 = xe[tw] - xe[tw+1]    V1 = xo[tw] + xe[tw+1]
      V2 = xe[tw+1] - xo[tw]    V3 = xo[tw] - xo[tw+1]
    (4 ops of [128, 34x16] bf16 each, 2x mode)
  - PE: per (oc chunk, sample): one PSUM tile [128, 4pw, 512] (4 banks),
    48 matmuls of [128x128] @ [128, 32h x 16tw] accumulating over
    (ic, kh); the kh shifts are strided APs into V's 34-row buffer.
    Two tiles ping-pong across the 8 PSUM banks so eviction never
    stalls the PE.  192 matmuls x 512 cols total.
  - DVE inverse transform (psum fp32, 4 ops per fill):
      out[:, 2tw]   = M0 + M1 + M2
      out[:, 2tw+1] = M1 - M2 - M3
  - Scalar (activation) engine applies the demod scale (per-partition
    scale AP), then the result DMAs out.  Host-side weight transform:
    Wt = G W (G = [[1,0,0],[.5,.5,.5],[.5,-.5,.5],[0,0,1]]) along w.
"""

import os

import numpy as np

N, C_IN, H, W = 8, 512, 32, 32
C_OUT, K = 512, 3
EPS = 1e-08
HP = H + 2  # 34 padded rows
WE = 17  # deinterleaved (even/odd) padded column count
TW = 16  # w-tiles per row
PW = 4  # Winograd F(2,3) transform length
IC = C_IN // 128  # 4 input-channel chunks
S = 2  # samples per core
OCC = 2  # out-channel chunks of 128 per core (256 of 512)
NCORES = 8


def _build_bass():
    import concourse.bass as bass  # noqa: F401
    import concourse.mybir as mybir
    import concourse.tile as tile
    from concourse import bacc

    f32 = mybir.dt.float32
    bf16 = mybir.dt.bfloat16

    nc = bacc.Bacc("TRN2")

    # [p=ci%128, s, ic, h, we] even / odd padded input columns
    xe_d = nc.dram_tensor("xe", [128, S, IC, HP, WE], bf16, kind="ExternalInput")
    xo_d = nc.dram_tensor("xo", [128, S, IC, HP, WE], bf16, kind="ExternalInput")
    # [p=ci%128, oc, ic, kh, pw, co] transformed weights (consumption order)
    wt_d = nc.dram_tensor("wt", [128, OCC, IC, K, PW, 128], bf16, kind="ExternalInput")
    # [p=o%128, oc, s] demod scale
    sc_d = nc.dram_tensor("sc", [128, OCC, S], f32, kind="ExternalInput")
    # [s, oc, p=o%128, pix] scaled conv output
    out_d = nc.dram_tensor("out", [S, OCC, 128, H * W], f32, kind="ExternalOutput")

    with tile.TileContext(nc) as tc:
        with (
            tc.tile_pool(name="singles", bufs=1) as singles,
            tc.tile_pool(name="psum", bufs=2, space="PSUM") as psum,
            tc.tile_pool(name="tmps", bufs=2) as tmps,
            tc.tile_pool(name="outs", bufs=2) as outs,
        ):
            sc_s = singles.tile([128, OCC, S], f32)
            nc.gpsimd.dma_start(out=sc_s, in_=sc_d[:])

            # ---- input DMA, paced + in consumption order ----
            from concourse.tile_rust import add_dep_helper

            CONC = int(os.environ.get("CONV_DMA_CONC", "4"))
            dma_chain = []

            def chain_dma(out, in_):
                eng = (nc.sync, nc.scalar)[len(dma_chain) % 2]
                bi = eng.dma_start(out=out, in_=in_)
                i = len(dma_chain)
                if i >= CONC:
                    add_dep_helper(
                        bi.ins,
                        dma_chain[i - CONC].ins,
                        sync=True,
                        reason="dma pacing",
                    )
                dma_chain.append(bi)

            xe_s = singles.tile([128, S, IC, HP, WE], bf16, name="xe")
            xo_s = singles.tile([128, S, IC, HP, WE], bf16, name="xo")
            wt_s = singles.tile([128, OCC, IC, K, PW, 128], bf16, name="wt")

            # x chunks first (gate the transforms), in (s, ic) order
            for s in range(S):
                for ic in range(IC):
                    chain_dma(xe_s[:, s, ic], xe_d[:, s, ic])
                    chain_dma(xo_s[:, s, ic], xo_d[:, s, ic])
            # weights in matmul consumption order: (oc, (s), ic, kh, pw)
            for oc in range(OCC):
                for ic in range(IC):
                    for kh in range(K):
                        chain_dma(wt_s[:, oc, ic, kh], wt_d[:, oc, ic, kh])

            # ---- DVE: F(2,3) data transform -> V[p, s, ic, pw, h, tw] ----
            v_s = singles.tile([128, S, IC, PW, HP, TW], bf16, name="v")
            for s in range(S):
                for ic in range(IC):
                    e0 = xe_s[:, s, ic, :, 0:TW]
                    e1 = xe_s[:, s, ic, :, 1 : TW + 1]
                    o0 = xo_s[:, s, ic, :, 0:TW]
                    o1 = xo_s[:, s, ic, :, 1 : TW + 1]
                    nc.vector.tensor_sub(v_s[:, s, ic, 0], e0, e1)
                    nc.vector.tensor_add(v_s[:, s, ic, 1], o0, e1)
                    nc.vector.tensor_sub(v_s[:, s, ic, 2], e1, o0)
                    nc.vector.tensor_sub(v_s[:, s, ic, 3], o0, o1)

            # ---- PE fills + DVE inverse + Act scale + out DMA ----
            for oc in range(OCC):
                for s in range(S):
                    ps = psum.tile([128, PW, H * TW], f32, tag="ps", name="ps")
                    for ic in range(IC):
                        for kh in range(K):
                            for pw in range(PW):
                                nc.tensor.matmul(
                                    ps[:, pw, :],
                                    wt_s[:, oc, ic, kh, pw, :],
                                    v_s[:, s, ic, pw, kh : kh + H, :],
                                    start=(ic == 0 and kh == 0),
                                    stop=(ic == IC - 1 and kh == K - 1),
                                )
                    # inverse transform: unscaled out halves (even/odd w)
                    last = oc == OCC - 1 and s == S - 1
                    o_u = outs.tile([128, H, W], f32, tag="o_u", name="o_u")
                    o_f = outs.tile([128, H * W], f32, tag="o_f", name="o_f")
                    # h-split the last fill's eviction for a shorter tail
                    hblocks = (
                        [(0, H // 2), (H // 2, H)] if last else [(0, H)]
                    )
                    for h0, h1 in hblocks:
                        nh = h1 - h0
                        a = tmps.tile([128, H * TW], f32, tag="a", name="a")
                        t = tmps.tile([128, H * TW], f32, tag="t", name="t")
                        u = tmps.tile([128, H * TW], f32, tag="u", name="u")
                        m = [
                            ps[:, pw, h0 * TW : h1 * TW] for pw in range(PW)
                        ]
                        av = a[:, h0 * TW : h1 * TW]
                        tv = t[:, h0 * TW : h1 * TW]
                        uv = u[:, h0 * TW : h1 * TW]
                        # DVE has one PSUM read port: stage M1 to SBUF via
                        # the activation engine so no op reads PSUM twice.
                        nc.scalar.copy(av, m[1])
                        nc.vector.tensor_add(tv, av, m[0])
                        nc.vector.tensor_add(o_u[:, h0:h1, 0::2], tv, m[2])
                        nc.vector.tensor_sub(uv, av, m[2])
                        nc.vector.tensor_sub(o_u[:, h0:h1, 1::2], uv, m[3])
                        # demod scale on the otherwise-idle activation engine
                        nc.scalar.mul(
                            o_f[:, h0 * W : h1 * W],
                            o_u[:, h0:h1, :],
                            sc_s[:, oc, s : s + 1],
                        )
                        nc.sync.dma_start(
                            out=out_d[s, oc, :, h0 * W : h1 * W],
                            in_=o_f[:, h0 * W : h1 * W],
                        )

    nc.finalize()
    return nc


def _prep_host(x: np.ndarray, y: np.ndarray, weight: np.ndarray):
    """Shard + lay out inputs for the 8 cores. Returns per-core input maps."""
    import ml_dtypes

    bf16 = ml_dtypes.bfloat16

    # demod scale, matching the fp32 reference math
    s = y + 1.0  # [N, O]
    wsq = np.sum(weight * weight, axis=(1, 2, 3))  # [O]
    scale = s / np.sqrt(wsq[None, :] * (s * s) + EPS)  # [N, O]

    # edge-replicate pad -> [N, C, 34, 34]; deinterleave columns
    xp = np.pad(x, ((0, 0), (0, 0), (1, 1), (1, 1)), mode="edge")
    xe = np.ascontiguousarray(xp[:, :, :, 0::2]).astype(bf16)  # [N, C, 34, 17]
    xo = np.ascontiguousarray(xp[:, :, :, 1::2]).astype(bf16)

    # F(2,3) weight transform along w: Wt[pw, o, i, kh] = (G W)[pw]
    g0, g1, g2 = weight[..., 0], weight[..., 1], weight[..., 2]  # [O, I, 3]
    wt = np.stack(
        [g0, (g0 + g1 + g2) * 0.5, (g0 - g1 + g2) * 0.5, g2], axis=0
    ).astype(bf16)  # [PW, O, I, K]

    in_maps = []
    for c in range(NCORES):
        g, oh = c // 2, c % 2
        ns = slice(2 * g, 2 * g + 2)
        os_ = slice(oh * 256, oh * 256 + 256)
        # [s, ic, p, h, we] -> [p, s, ic, h, we]
        xec = xe[ns].reshape(S, IC, 128, HP, WE).transpose(2, 0, 1, 3, 4)
        xoc = xo[ns].reshape(S, IC, 128, HP, WE).transpose(2, 0, 1, 3, 4)
        # wt[pw, o, i, kh] -> [pw, oc, co, ic, p, kh] -> [p, oc, ic, kh, pw, co]
        wtc = wt[:, os_].reshape(PW, OCC, 128, IC, 128, K).transpose(4, 1, 3, 5, 0, 2)
        # scale -> [p, oc, s]
        scc = scale[ns, os_].reshape(S, OCC, 128).transpose(2, 1, 0)
        in_maps.append(
            {
                "xe": np.ascontiguousarray(xec),
                "xo": np.ascontiguousarray(xoc),
                "wt": np.ascontiguousarray(wtc),
                "sc": np.ascontiguousarray(scc.astype(np.float32)),
            }
        )
    return in_maps


def _gather(results) -> np.ndarray:
    out = np.empty((N, C_OUT, H, W), np.float32)
    for c in range(NCORES):
        g, oh = c // 2, c % 2
        r = results[c]["out"].reshape(S, OCC, 128, H, W)
        for s in range(S):
            for oc in range(OCC):
                out[2 * g + s, oh * 256 + oc * 128 : oh * 256 + oc * 128 + 128] = r[
                    s, oc
                ]
    return out


def kernel(x: np.ndarray, y: np.ndarray, weight: np.ndarray) -> np.ndarray:
    from concourse.bass_utils import run_bass_kernel_spmd

    x = np.asarray(x, dtype=np.float32)
    y = np.asarray(y, dtype=np.float32)
    weight = np.asarray(weight, dtype=np.float32)

    in_maps = _prep_host(x, y, weight)
    nc = _build_bass()
    results = run_bass_kernel_spmd(nc, in_maps, core_ids=list(range(NCORES))).results
    return _gather(results)
